# revision 12
# baseline (speedup 1.0000x reference)
"""DPPConv2d Trainium2 Bass kernel.

Reference computation (per sample s):
  pooled = mean_{h,w} x[s]                              [Cin]
  h      = relu(pooled @ W1.T)                          [hidden]
  logits = h @ W2.T + b2                                [P*Cout]
  attn   = softmax(logits.reshape(P, Cout) / 0.5, p)    [P, Cout]
  m      = (mean_{o,i}(|W[p,:,:,k,l]| - thr[p,:]) > 0)  [P, K, K]
  agg    = sum_p attn[p, co] * m[p, kl] * W[p, co, ci, kl]
  out[s] = conv2d(x[s], agg, pad=1)                     [Cout, H, W]

Sharding: data-parallel over batch -- 8 cores x 4 samples each; weight
bank / psa weights / threshold replicated on every core.

Design (steady state ~67us/pass per core, PE-roofline bound: the 288
per-pass conv matmuls are ~61us at the f32r full rate):
  - conv as 9 shifted accumulating f32r matmuls per 8-row output chunk
    (self-loading encoding: no per-matmul Ldweights instruction, unlike
    bf16); x staged in 6 rotating f32r SBUF slots so the next rep's
    input DMAs overlap this rep's compute.
  - output staged as bf16 and DMA'd to a bf16 DRAM tensor (halves
    output HBM traffic); the host upcasts to f32.
  - pooling via tensor_scalar accum_out (fast DVE mode) instead of
    TensorReduce (which has no fast mode).
  - SE logits/softmax computed directly in transposed [cout, p, s]
    layout via tiny per-pattern matmuls (bias via the exp activation's
    per-partition bias operand, pre-scaled host-side), so no PE
    transposes sit on the softmax -> aggregation critical path.
  - software pipelining across reps: each rep's DMAs, pooling, mask,
    and SE are emitted inside the previous rep's conv phase, with
    wm/attn/pooled double-buffered by rep parity so the hoisted writes
    never race this rep's readers in program order; each sample's
    aggregation+transposes are emitted one sample ahead of its convs.
"""

import os
import sys

try:
    import concourse.bass as bass  # noqa: F401
except Exception:  # pragma: no cover
    sys.path.insert(0, "/opt/trn_rl_repo")

from contextlib import ExitStack

import numpy as np

import concourse.bass as bass
import concourse.tile as tile
from concourse import mybir
from concourse.bass_utils import run_bass_kernel_spmd

N_CORES = 8
BS = 32
BS_LOCAL = BS // N_CORES  # 4
CIN = 128
COUT = 128
H = W = 64
P_PAT = 4
KS = 3
KK = KS * KS
HID = 33
TEMP = 0.5
YC = 8          # output rows per conv chunk
N_CHUNK = H // YC

F32 = mybir.dt.float32
F32R = mybir.dt.float32r
BF16 = mybir.dt.bfloat16


def build_nc(use_f32r=True, rep=1, mm=None):
    if mm is None:
        mm = "f32r"
    mm_dt = {"f32r": F32R, "f32": F32, "bf16": BF16}[mm]
    n_xslot = 8 if mm_dt == BF16 else 6

    nc = bass.Bass("TRN2", target_bir_lowering=False, debug=False,
                   num_swdge_queues=4)

    x_d = nc.dram_tensor("x", [BS_LOCAL, CIN, H + 2, W + 2], F32, kind="ExternalInput")
    w1_d = nc.dram_tensor("psa_w1", [HID, CIN], F32, kind="ExternalInput")
    w2_d = nc.dram_tensor("psa_w2", [P_PAT * COUT, HID], F32, kind="ExternalInput")
    b2_d = nc.dram_tensor("psa_b2", [COUT, P_PAT], F32, kind="ExternalInput")
    w_d = nc.dram_tensor("weight", [P_PAT, COUT, CIN, KS, KS], F32, kind="ExternalInput")
    thr_d = nc.dram_tensor("threshold", [P_PAT, COUT], F32, kind="ExternalInput")
    id_d = nc.dram_tensor("ident", [128, 128], F32, kind="ExternalInput")
    out_d = nc.dram_tensor("out", [BS_LOCAL, COUT, H, W], BF16, kind="ExternalOutput")

    with tile.TileContext(nc) as tc, ExitStack() as ctx:
        consts = ctx.enter_context(tc.tile_pool(name="consts", bufs=1))
        wpool = ctx.enter_context(tc.tile_pool(name="wpool", bufs=1))
        xpool = ctx.enter_context(tc.tile_pool(name="xpool", bufs=1))
        aggp = ctx.enter_context(tc.tile_pool(name="aggp", bufs=2))
        lhsp = ctx.enter_context(tc.tile_pool(name="lhsp", bufs=2))
        outp = ctx.enter_context(tc.tile_pool(name="outp", bufs=4))
        ps_small = ctx.enter_context(
            tc.tile_pool(name="ps_small", bufs=2, space="PSUM"))
        ps_tp = ctx.enter_context(
            tc.tile_pool(name="ps_tp", bufs=2, space="PSUM"))
        ps_mm = ctx.enter_context(
            tc.tile_pool(name="ps_mm", bufs=4, space="PSUM"))

        # ---- persistent tiles -------------------------------------------
        ident = consts.tile([128, 128], F32, tag="ident")
        nc.sync.dma_start(ident[:], id_d[:])
        ones_col = consts.tile([128, 1], F32, tag="ones_col")
        nc.vector.memset(ones_col[:], 1.0)
        ones_row = consts.tile([1, 128], F32, tag="ones_row")
        nc.vector.memset(ones_row[:], 1.0)

        w1_sb = consts.tile([HID, CIN], F32, tag="w1_sb")
        w1T = consts.tile([CIN, HID], F32, tag="w1T")
        w2_raw = consts.tile([128, P_PAT, HID], F32, tag="w2_raw")
        w2b = consts.tile([HID, P_PAT * COUT], F32, tag="w2b")
        # b2 pre-transposed to [co, p] and pre-scaled by 1/TEMP on the host
        b2s = consts.tile([COUT, P_PAT], F32, tag="b2s")
        thr_T = consts.tile([COUT, P_PAT], F32, tag="thr_T")
        pool_scr = consts.tile([CIN, (H + 2) * (W + 2)], mm_dt, tag="pool_scr")
        h_sb = consts.tile([HID, BS_LOCAL], F32, tag="h_sb")
        wsum = consts.tile([128, P_PAT, KK], F32, tag="wsum")
        thr_sc = consts.tile([1, P_PAT], F32, tag="thr_sc")
        z_row = consts.tile([1, P_PAT * KK], F32, tag="z_row")
        m_row = consts.tile([1, P_PAT * KK], F32, tag="m_row")
        mb = consts.tile([128, P_PAT * KK], F32, tag="mb")
        # SE tensors in transposed [co, p, s] layout (no PE transposes on
        # the softmax path); attn/pooled/wm double-buffered by rep parity
        # so the software-pipelined next-rep writes never race this rep's
        # readers in program order.
        sm_e = consts.tile([COUT, P_PAT, BS_LOCAL], F32, tag="sm_e")
        sm_sum = consts.tile([COUT, BS_LOCAL], F32, tag="sm_sum")
        sm_rec = consts.tile([COUT, BS_LOCAL], F32, tag="sm_rec")
        pooled2 = [consts.tile([CIN, BS_LOCAL], F32, tag=f"pooled{i}",
                               name=f"pooled{i}")
                   for i in range(2)]
        attn_T2 = [consts.tile([COUT, P_PAT, BS_LOCAL], F32, tag=f"attn_T{i}",
                               name=f"attn_T{i}")
                   for i in range(2)]

        wco = wpool.tile([128, P_PAT, CIN, KK], F32, tag="wco")
        wm2 = [wpool.tile([128, P_PAT, CIN * KK], F32, tag=f"wm{i}",
                          name=f"wm{i}")
               for i in range(2)]
        xs = [xpool.tile([CIN, H + 2, W + 2], mm_dt, tag=f"xs{i}", name=f"xs{i}")
              for i in range(n_xslot)]

        def xr(r, s):
            return xs[(r * BS_LOCAL + s) % n_xslot]

        def emit_dma(r):
            # weights first so the (early, small) mask/SE stages of the
            # next rep unblock before the bulk x transfers finish
            for p in range(P_PAT):
                nc.sync.dma_start(
                    wco[:, p], w_d[p].rearrange("co ci k l -> co ci (k l)"))
            nc.sync.dma_start(w1_sb[:], w1_d[:])
            nc.sync.dma_start(
                w2_raw[:], w2_d[:].rearrange("(c p) h -> p c h", p=128))
            nc.sync.dma_start(b2s[:], b2_d[:])
            nc.sync.dma_start(thr_T[:], thr_d[:].rearrange("p co -> co p"))
            # x zero-padded to 66x66 f32 on the host; the gpsimd SWDGE
            # cast-DMA downconverts to bf16 on the fly
            for s in range(BS_LOCAL):
                nc.gpsimd.dma_start(xr(r, s)[:], x_d[s])

        def emit_pooling(r):
            # pooled sums via tensor_scalar accumulate (fast DVE mode);
            # mean folded into the relu activation scale
            pooled = pooled2[r % 2]
            for s in range(BS_LOCAL):
                nc.vector.tensor_scalar(
                    pool_scr[:], xr(r, s)[:].rearrange("c h w -> c (h w)"),
                    1.0, None, op0=mybir.AluOpType.mult,
                    op1=mybir.AluOpType.add,
                    accum_out=pooled[:, s:s + 1])

        def emit_mask_fold(r):
            # binary spatial mask, folded into the weight bank
            for p in range(P_PAT):
                nc.vector.reduce_sum(
                    wsum[:, p], wco[:, p].rearrange("co ci kl -> co kl ci"),
                    axis=mybir.AxisListType.X, apply_absolute_value=True)
            wsum_ps = ps_small.tile([1, P_PAT * KK], F32, tag="sm")
            nc.tensor.matmul(
                wsum_ps[:], ones_col[:], wsum[:].rearrange("co p kl -> co (p kl)"))
            thr_ps = ps_small.tile([1, P_PAT], F32, tag="sm")
            nc.tensor.matmul(thr_ps[:], ones_col[:], thr_T[:])
            nc.vector.tensor_scalar_mul(thr_sc[:], thr_ps[:], 1.0 / COUT)
            for p in range(P_PAT):
                nc.vector.tensor_scalar(
                    z_row[0:1, p * KK:(p + 1) * KK],
                    wsum_ps[0:1, p * KK:(p + 1) * KK],
                    1.0 / (COUT * CIN), thr_sc[0:1, p:p + 1],
                    op0=mybir.AluOpType.mult, op1=mybir.AluOpType.subtract)
            nc.vector.tensor_scalar(
                m_row[:], z_row[:], 0.0, None, op0=mybir.AluOpType.is_gt)
            mb_ps = ps_small.tile([128, P_PAT * KK], F32, tag="sm")
            nc.tensor.matmul(mb_ps[:], ones_row[:], m_row[:])
            nc.vector.tensor_copy(mb[:], mb_ps[:])

            wm = wm2[r % 2]
            for p in range(P_PAT):
                for kl in range(KK):
                    nc.vector.tensor_scalar_mul(
                        wm[:, p].rearrange("co (ci kl) -> co ci kl", kl=KK)[:, :, kl],
                        wco[:, p, :, kl],
                        mb[:, p * KK + kl:p * KK + kl + 1])

        def emit_se_front(r):
            # SE attention MLP (batched over the 4 local samples)
            w1T_ps = ps_small.tile([CIN, HID], F32, tag="sm")
            nc.tensor.transpose(w1T_ps[:], w1_sb[:], ident[0:HID, 0:HID])
            nc.scalar.copy(w1T[:], w1T_ps[:])

            h_ps = ps_small.tile([HID, BS_LOCAL], F32, tag="sm")
            nc.tensor.matmul(h_ps[:], w1T[:], pooled2[r % 2][:])
            nc.scalar.activation(
                h_sb[:], h_ps[:], mybir.ActivationFunctionType.Relu,
                scale=1.0 / (H * W))

            for c in range(P_PAT):
                w2T_ps = ps_small.tile([HID, 128], F32, tag="sm")
                nc.tensor.transpose(w2T_ps[:], w2_raw[:, c], ident[:])
                nc.scalar.copy(w2b[:, c * 128:(c + 1) * 128], w2T_ps[:])

        def emit_se_back(r):
            # logits computed directly in transposed [co, s] layout: one
            # tiny matmul per pattern, so no PE transposes sit on the
            # softmax -> aggregation critical path.
            attn_T = attn_T2[r % 2]
            lgT_ps = ps_small.tile([COUT, P_PAT, BS_LOCAL], F32, tag="sm")
            for p in range(P_PAT):
                nc.tensor.matmul(
                    lgT_ps[:, p], w2b[:, p * COUT:(p + 1) * COUT], h_sb[:])
            # softmax over p (temperature 0.5 -> scale 2.0). No max
            # subtraction: |logits|/temp is bounded far below exp overflow.
            for p in range(P_PAT):
                nc.scalar.activation(
                    sm_e[:, p], lgT_ps[:, p], mybir.ActivationFunctionType.Exp,
                    bias=b2s[:, p:p + 1], scale=1.0 / TEMP)
            nc.vector.tensor_add(sm_sum[:], sm_e[:, 0], sm_e[:, 1])
            nc.vector.tensor_add(sm_sum[:], sm_sum[:], sm_e[:, 2])
            nc.vector.tensor_add(sm_sum[:], sm_sum[:], sm_e[:, 3])
            nc.vector.reciprocal(sm_rec[:], sm_sum[:])
            for p in range(P_PAT):
                nc.vector.tensor_mul(attn_T[:, p], sm_e[:, p], sm_rec[:])

        lhsT_of = {}

        def emit_prep(r, s):
            # weighted kernel aggregation (DVE; the Pool engine only
            # accepts DMA instructions through this walrus build)
            wm = wm2[r % 2]
            attn_T = attn_T2[r % 2]
            eng = nc.vector
            agg = aggp.tile([128, CIN, KK], F32, tag="agg")
            eng.tensor_scalar_mul(
                agg[:].rearrange("co ci kl -> co (ci kl)"), wm[:, 0],
                attn_T[:, 0, s:s + 1])
            for p in range(1, P_PAT):
                eng.scalar_tensor_tensor(
                    agg[:].rearrange("co ci kl -> co (ci kl)"),
                    wm[:, p], attn_T[:, p, s:s + 1],
                    agg[:].rearrange("co ci kl -> co (ci kl)"),
                    op0=mybir.AluOpType.mult, op1=mybir.AluOpType.add)

            # transpose [co,ci] -> [ci,co] per spatial tap; grouped into
            # 4+4+1 PSUM tiles so the SBUF copies are 3 wide ops
            lhsT = lhsp.tile([CIN, KK, COUT], mm_dt, tag="lhsT")
            lhsT_of[(r, s)] = lhsT
            for g0, gn in ((0, 4), (4, 4), (8, 1)):
                tp_ps = ps_tp.tile([CIN, 4, COUT], F32, tag="tp_ps")
                for j in range(gn):
                    nc.tensor.transpose(
                        tp_ps[:, j], agg[:, :, g0 + j], ident[:])
                nc.scalar.copy(lhsT[:, g0:g0 + gn], tp_ps[:, 0:gn])

        def emit_convs(r, s, defer_last=False, mid_hook=None):
            lhsT = lhsT_of.pop((r, s))
            deferred = None
            for yc in range(N_CHUNK):
                if yc == 3 and mid_hook is not None:
                    # splice the next rep's sample-0 prep between chunks so
                    # its transposes/copies complete well before the rep
                    # boundary (deps are ready by now; emitting earlier
                    # would overflow the 4-deep PE wait queue)
                    mid_hook()
                y0 = yc * YC
                pt = ps_mm.tile([COUT, YC, W], F32, tag="pt")
                for i, (dk, dl) in enumerate(
                        (dk, dl) for dk in range(KS) for dl in range(KS)):
                    nc.tensor.matmul(
                        pt[:],
                        lhsT[:, dk * KS + dl],
                        xr(r, s)[:, y0 + dk:y0 + dk + YC, dl:dl + W],
                        start=(i == 0), stop=(i == KK - 1))

                ot = outp.tile([COUT, YC, W], BF16, tag="ot")

                def _tail(pt=pt, ot=ot, s=s, y0=y0):
                    nc.scalar.copy(ot[:], pt[:])
                    nc.sync.dma_start(out_d[s, :, y0:y0 + YC, :], ot[:])

                if defer_last and yc == N_CHUNK - 1:
                    # emitted by the caller after the next rep's first lhsT
                    # copies, so those aren't queued behind this one on Act
                    deferred = _tail
                else:
                    _tail()
            return deferred

        # Software pipeline across reps: each rep's input DMAs, pooling,
        # mask+fold and SE are emitted inside the PREVIOUS rep's conv
        # phase, so the serial attention->aggregation chain executes
        # during the previous rep's matmuls instead of stalling the PE at
        # the rep boundary. wm/attn_T/pooled are double-buffered by rep
        # parity, so the next-rep writes emitted here cannot race this
        # rep's readers in program order. The handful of hoisted PE
        # instructions whose inputs aren't ready yet park in the PE wait
        # queue (depth 4) while the conv matmuls behind them keep issuing;
        # the emission points below keep the parked count within depth.
        emit_dma(0)
        emit_pooling(0)
        emit_mask_fold(0)
        emit_se_front(0)
        emit_se_back(0)
        emit_prep(0, 0)
        emit_prep(0, 1)
        for r in range(rep):
            emit_convs(r, 0)
            if r + 1 < rep:
                emit_dma(r + 1)
            emit_prep(r, 2)
            emit_convs(r, 1)
            if r + 1 < rep:
                emit_mask_fold(r + 1)
            emit_prep(r, 3)
            emit_convs(r, 2)
            if r + 1 < rep:
                emit_pooling(r + 1)
                emit_se_front(r + 1)
                emit_se_back(r + 1)
            hook = (lambda: emit_prep(r + 1, 0)) if r + 1 < rep else None
            emit_convs(r, 3, mid_hook=hook)
            if r + 1 < rep:
                emit_prep(r + 1, 1)

    _split_excess_waits(nc)
    return nc


def _split_excess_waits(nc, max_inline=1):
    """Hoist extra sync waits into standalone EventSemaphore instructions.

    This walrus build rejects instructions whose encoded sync-command
    count exceeds the ISA struct capacity ("Too many sync wait
    commands") -- in practice more than one wait per compute
    instruction. Engines execute their instruction stream in order, so
    blocking on a preceding same-engine EventSemaphore is equivalent to
    the instruction carrying the wait itself.
    """
    n = 0
    for f in nc.m.functions:
        for blk in f.blocks:
            out = []
            for inst in blk.instructions:
                si = inst.sync_info
                if si is not None and len(si.on_wait) > max_inline:
                    waits = list(si.on_wait)
                    keep = waits[:max_inline]
                    for w in waits[max_inline:]:
                        n += 1
                        ev = mybir.InstEventSemaphore(
                            name=f"WSPLIT-{n}", ins=[], outs=[])
                        ev.engine = inst.engine
                        ev.sync_info = mybir.SyncInfo(on_wait=[w], on_update=[])
                        ev.debug = inst.debug
                        nc.inst_map[ev.name] = ev
                        out.append(ev)
                    inst.sync_info = mybir.SyncInfo(
                        on_wait=keep, on_update=list(si.on_update))
                out.append(inst)
            blk.instructions = out
    return n


class _Runner:
    """Cached PJRT executor for the 8-core SPMD program.

    Mirrors bass2jax.run_bass_via_pjrt's multi-core path but keeps the
    jitted shard_map callable (and the device mesh) alive across calls,
    so repeat invocations skip retracing and recompilation.
    """

    def __init__(self, nc):
        import jax
        import jax.numpy as jnp
        from jax.experimental.shard_map import shard_map
        from jax.sharding import Mesh, PartitionSpec, NamedSharding
        from concourse import bass2jax, mybir as _mb

        bass2jax.install_neuronx_cc_hook()
        self.jax = jax
        self.nc = nc
        assert nc.dbg_addr is None

        partition_name = (nc.partition_id_tensor.name
                          if nc.partition_id_tensor else None)
        in_names, out_names, out_avals, zero_shapes = [], [], [], []
        for alloc in nc.m.functions[0].allocations:
            if not isinstance(alloc, _mb.MemoryLocationSet):
                continue
            name = alloc.memorylocations[0].name
            if alloc.kind == "ExternalInput":
                if name != partition_name:
                    in_names.append(name)
            elif alloc.kind == "ExternalOutput":
                out_names.append(name)
                shape = tuple(alloc.tensor_shape)
                dtype = _mb.dt.np(alloc.dtype)
                out_avals.append(jax.core.ShapedArray(shape, dtype))
                zero_shapes.append((shape, dtype))
        self.in_names = list(in_names)
        self.out_names = out_names
        self.out_avals = out_avals
        n_params = len(in_names)
        n_outs = len(out_names)
        all_in_names = in_names + out_names
        if partition_name is not None:
            all_in_names.append(partition_name)
        donate = tuple(range(n_params, n_params + n_outs))

        def _body(*args):
            operands = list(args)
            if partition_name is not None:
                operands.append(bass2jax.partition_id_tensor())
            outs = bass2jax._bass_exec_p.bind(
                *operands,
                out_avals=tuple(out_avals),
                in_names=tuple(all_in_names),
                out_names=tuple(out_names),
                lowering_input_output_aliases=(),
                sim_require_finite=True,
                sim_require_nnan=True,
                nc=nc,
            )
            return tuple(outs)

        devices = jax.devices()[:N_CORES]
        self.mesh = Mesh(np.asarray(devices), ("core",))
        self.sharding = NamedSharding(self.mesh, PartitionSpec("core"))
        in_specs = (PartitionSpec("core"),) * (n_params + n_outs)
        out_specs = (PartitionSpec("core"),) * n_outs
        self.sharded = jax.jit(
            shard_map(_body, mesh=self.mesh, in_specs=in_specs,
                      out_specs=out_specs, check_rep=False),
            donate_argnums=donate, keep_unused=True)
        self._zero_makers = [
            jax.jit(
                (lambda sh=sh, dt=dt: jnp.zeros((N_CORES * sh[0],) + sh[1:], dt)),
                out_shardings=self.sharding)
            for sh, dt in zero_shapes
        ]

    def put_inputs(self, in_maps):
        """Concat per-core inputs on axis 0 and upload sharded."""
        cat = [
            np.concatenate([np.asarray(m[name]) for m in in_maps], axis=0)
            for name in self.in_names
        ]
        return [self.jax.device_put(a, self.sharding) for a in cat]

    def run(self, dev_inputs):
        zeros = [zm() for zm in self._zero_makers]
        outs = self.sharded(*dev_inputs, *zeros)
        self.jax.block_until_ready(outs)
        return outs

    def results(self, outs):
        res = []
        for c in range(N_CORES):
            res.append({
                name: np.asarray(outs[i]).reshape(
                    N_CORES, *self.out_avals[i].shape)[c]
                for i, name in enumerate(self.out_names)
            })
        return res


_RUNNER_CACHE = {}


def _get_runner(use_f32r=True, rep=1, mm=None):
    mm = mm or os.environ.get("DPP_MM", "f32r")
    key = (use_f32r, rep, mm)
    if key not in _RUNNER_CACHE:
        _RUNNER_CACHE[key] = _Runner(_get_nc(use_f32r=use_f32r, rep=rep, mm=mm))
    return _RUNNER_CACHE[key]


_NC_CACHE = {}


def _get_nc(use_f32r=True, rep=1, mm=None):
    mm = mm or os.environ.get("DPP_MM", "f32r")
    key = (use_f32r, rep, mm)
    if key not in _NC_CACHE:
        _NC_CACHE[key] = build_nc(use_f32r=use_f32r, rep=rep, mm=mm)
    return _NC_CACHE[key]


def make_in_maps(x, psa_w1, psa_w2, psa_b2, weight, threshold):
    x = np.asarray(x, dtype=np.float32)
    xp = np.zeros((BS, CIN, H + 2, W + 2), np.float32)
    xp[:, :, 1:H + 1, 1:W + 1] = x
    thr2 = np.ascontiguousarray(
        np.asarray(threshold, dtype=np.float32).reshape(P_PAT, COUT))
    b2s = np.asarray(psa_b2, np.float32).reshape(P_PAT, COUT).T / TEMP
    common = {
        "psa_w1": np.ascontiguousarray(np.asarray(psa_w1, np.float32)),
        "psa_w2": np.ascontiguousarray(np.asarray(psa_w2, np.float32)),
        "psa_b2": np.ascontiguousarray(b2s),
        "weight": np.ascontiguousarray(np.asarray(weight, np.float32)),
        "threshold": thr2,
        "ident": np.eye(128, dtype=np.float32),
    }
    return [
        {"x": xp[c * BS_LOCAL:(c + 1) * BS_LOCAL], **common}
        for c in range(N_CORES)
    ]


def kernel(x, psa_w1, psa_w2, psa_b2, weight, threshold):
    mm = os.environ.get("DPP_MM", "f32r")
    in_maps = make_in_maps(x, psa_w1, psa_w2, psa_b2, weight, threshold)
    res = None
    for attempt in range(2):
        try:
            r = _get_runner(mm=mm)
            outs = r.run(r.put_inputs(in_maps))
            res = r.results(outs)
            break
        except Exception:
            # transient device wedge (NRT_EXEC_UNIT_UNRECOVERABLE) usually
            # clears on retry; drop the cached runner so jax re-inits
            _RUNNER_CACHE.clear()
    if res is None:
        nc = _get_nc(mm=mm)
        res = run_bass_kernel_spmd(nc, in_maps, list(range(N_CORES))).results
    return np.concatenate(
        [np.asarray(res[c]["out"]).astype(np.float32) for c in range(N_CORES)],
        axis=0)
